# revision 26
# baseline (speedup 1.0000x reference)
"""Trainium2 Bass kernel for EnhancedLiLTRelationExtraction.

Shapes: B=2, L=512, H=768.
Outputs: rel_logits (B,L,L,3), checkmark (B,L,4), reasoning (B,8), solution (B,8).

Sharding: 8 cores = batch(2) x left-token-blocks(4).  Each core computes a
(128, 512) block of the pair grid plus the per-token checkmark head for its
token slice; cores 0 and 4 also carry the (tiny) reasoning/solution heads
for their batch.

Per-core pair path (the heavy part):
  Rt[k, j] = right_proj(b)[j, k]       (H on partitions, 6 chunks of 128)
  Lt[k, i] = left_proj(b)[i, k] + b1   (fp32, feeds per-partition scalar)
  for each left token i:  h = relu(Rt + Lt[:, i])  (one fused DVE/ACT op
  per k-chunk), then PE contracts h with rel_w2 chunk into PSUM,
  4 left tokens concurrently via column tiling (tile_position).

All matmul operands are bf16 (PSUM accumulation fp32).  DVE producer units
use the identity relu(r+l) = max(r,-l) + l: the max is a single-op
tensor_scalar, and the Sum_k w2*l correction (rank-1, j-independent) is
computed exactly on-device as C = Lt^T w2 (outputs co4/co5) and added back
on the host during unshard.  ACT units compute relu(r+l) directly and need
no correction; ACT takes whole trailing k-chunks (1 or 2, alternating by
block) so the correction per token is a fixed chunk-prefix sum.
"""

import sys

if "/opt/trn_rl_repo" not in sys.path:
    sys.path.insert(0, "/opt/trn_rl_repo")

import numpy as np

B, L, H = 2, 512, 768
P = 128
KC = H // P            # 6 k-chunks
NIPC = L // 4          # 128 left tokens per core
NG = NIPC // 4         # 32 groups of 4 left tokens
NCORES = 8

# Pair-loop matmul operands (Rt, h, w2) in bf16: required for PE column
# tiling (fp32/f32r weights use 4 physical PE columns per logical column, so
# their matmul destination must start at PSUM partition 0) and enables the
# DVE 4x perf mode for the relu producer.  Projections/heads stay float32r.
PAIR_BF16 = True
# 4-way PE column tiling for the pair contraction matmuls.
COL_TILING = True
# benchmarking only: trace the whole kernel body this many times in one NEFF
REPEAT = 1

# engine cost estimates (ns, HW-measured) for static DVE/ACT load balancing
_DVE_UNIT = 202
_ACT_UNIT = 613
_DVE_COPY = 658
_ACT_COPY = 570


def _akc(ig):
    """number of trailing k-chunks ACT computes for block ig (1 or 2)."""
    return 2 if ig % 2 == 0 else 1


class _Balancer:
    def __init__(self):
        self.t = {"dve": 0.0, "act": 0.0}

    def pick(self, dve_cost, act_cost):
        if self.t["dve"] + dve_cost <= self.t["act"] + act_cost:
            self.t["dve"] += dve_cost
            return "dve"
        self.t["act"] += act_cost
        return "act"


def _build_program():
    import concourse.mybir as mybir
    from concourse import bacc
    from concourse.tile import TileContext

    f32 = mybir.dt.float32
    mdt = mybir.dt.bfloat16
    pdt = mybir.dt.bfloat16
    ADD = mybir.AluOpType.add
    MAX = mybir.AluOpType.max
    RELU = mybir.ActivationFunctionType.Relu

    nc = bacc.Bacc("TRN2", target_bir_lowering=False)

    # ---- inputs (all host-side pre-rearranged to partition-major) ----
    seq = nc.dram_tensor("seq", [P, KC, L], mdt, kind="ExternalInput")
    seqi = nc.dram_tensor("seqi", [P, KC, NIPC], mdt, kind="ExternalInput")
    cls = nc.dram_tensor("cls", [P, KC, 1], mdt, kind="ExternalInput")
    wlp = nc.dram_tensor("wlp", [P, KC * KC, P], mdt, kind="ExternalInput")
    wrp = nc.dram_tensor("wrp", [P, KC * KC, P], mdt, kind="ExternalInput")
    relb1 = nc.dram_tensor("relb1", [P, KC], f32, kind="ExternalInput")
    # rel_w2 padded from 3 to 32 output columns with zeros so each PE column
    # group writes its full 32-partition PSUM slice (no uninitialized reads).
    w2 = nc.dram_tensor("w2", [P, KC, 32], pdt, kind="ExternalInput")
    ckw1 = nc.dram_tensor("ckw1", [P, KC * 3, P], mdt, kind="ExternalInput")
    ckb1 = nc.dram_tensor("ckb1", [P, 3], f32, kind="ExternalInput")
    ckw2 = nc.dram_tensor("ckw2", [P, 3, 4], mdt, kind="ExternalInput")
    rsw1 = nc.dram_tensor("rsw1", [P, KC * KC, P], mdt, kind="ExternalInput")
    rsb1 = nc.dram_tensor("rsb1", [P, KC], f32, kind="ExternalInput")
    rsw2 = nc.dram_tensor("rsw2", [P, KC, 8], mdt, kind="ExternalInput")
    slw1 = nc.dram_tensor("slw1", [P, 2 * KC * KC, P], mdt, kind="ExternalInput")
    slb1 = nc.dram_tensor("slb1", [P, KC], f32, kind="ExternalInput")
    slw2 = nc.dram_tensor("slw2", [P, KC, 8], mdt, kind="ExternalInput")

    # ---- outputs ----
    relo = nc.dram_tensor("relo", [NG, P, L], f32, kind="ExternalOutput")
    cko = nc.dram_tensor("cko", [4, NIPC], f32, kind="ExternalOutput")
    rso = nc.dram_tensor("rso", [8, 1], f32, kind="ExternalOutput")
    slo = nc.dram_tensor("slo", [8, 1], f32, kind="ExternalOutput")
    co4 = nc.dram_tensor("co4", [NIPC, 32], f32, kind="ExternalOutput")
    co5 = nc.dram_tensor("co5", [NIPC, 32], f32, kind="ExternalOutput")

    with TileContext(nc) as tc:
      for _rep in range(REPEAT):
        bal = _Balancer()
        with (
            tc.tile_pool(name="const", bufs=1) as cpool,
            tc.tile_pool(name="rtlt", bufs=1) as rl_pool,
            tc.tile_pool(name="hpool", bufs=12) as hpool,
            tc.tile_pool(name="opool", bufs=4) as opool,
            tc.tile_pool(name="ppsum", bufs=2, space="PSUM") as ppsum,
            tc.tile_pool(name="pair_psum", bufs=4, space="PSUM") as pair_psum,
            tc.tile_pool(name="head_psum", bufs=2, space="PSUM") as head_psum,
        ):
            # ---- load constants ----
            def load(name, shape, dtype, src):
                t = cpool.tile(shape, dtype, name=name)
                nc.sync.dma_start(t, src)
                return t

            seq_sb = load("seq_sb", [P, KC, L], mdt, seq[:, :, :])
            seqi_sb = load("seqi_sb", [P, KC, NIPC], mdt, seqi[:, :, :])
            cls_sb = load("cls_sb", [P, KC, 1], mdt, cls[:, :, :])
            wlp_sb = load("wlp_sb", [P, KC * KC, P], mdt, wlp[:, :, :])
            wrp_sb = load("wrp_sb", [P, KC * KC, P], mdt, wrp[:, :, :])
            relb1_sb = load("relb1_sb", [P, KC], f32, relb1[:, :])
            w2_sb = load("w2_sb", [P, KC, 32], pdt, w2[:, :, :])
            ckw1_sb = load("ckw1_sb", [P, KC * 3, P], mdt, ckw1[:, :, :])
            ckb1_sb = load("ckb1_sb", [P, 3], f32, ckb1[:, :])
            ckw2_sb = load("ckw2_sb", [P, 3, 4], mdt, ckw2[:, :, :])
            rsw1_sb = load("rsw1_sb", [P, KC * KC, P], mdt, rsw1[:, :, :])
            rsb1_sb = load("rsb1_sb", [P, KC], f32, rsb1[:, :])
            rsw2_sb = load("rsw2_sb", [P, KC, 8], mdt, rsw2[:, :, :])
            slw1_sb = load("slw1_sb", [P, 2 * KC * KC, P], mdt, slw1[:, :, :])
            slb1_sb = load("slb1_sb", [P, KC], f32, slb1[:, :])
            slw2_sb = load("slw2_sb", [P, KC, 8], mdt, slw2[:, :, :])

            # ---- projections: Rt (k-part, j) and Lt (k-part, i) ----
            rt_sb = rl_pool.tile([P, KC, L], pdt, name="rt_sb")
            lt_sb = rl_pool.tile([P, KC, NIPC], f32, name="lt_sb")
            lt_bf = rl_pool.tile([P, KC, NIPC], pdt, name="lt_bf")
            lt_ng = rl_pool.tile([P, KC, NIPC], f32, name="lt_ng")
            for kc in range(KC):
                ps = ppsum.tile([P, L], f32, tag="proj", name="ps_rt")
                for hc in range(KC):
                    nc.tensor.matmul(
                        ps,
                        wrp_sb[:, hc * KC + kc, :],
                        seq_sb[:, hc, :],
                        start=(hc == 0),
                        stop=(hc == KC - 1),
                    )
                eng = bal.pick(_DVE_COPY, _ACT_COPY)
                if eng == "dve":
                    nc.vector.tensor_copy(rt_sb[:, kc, :], ps)
                else:
                    nc.scalar.copy(rt_sb[:, kc, :], ps)
            for kc in range(KC):
                ps = ppsum.tile([P, NIPC], f32, tag="proj", name="ps_lt")
                for hc in range(KC):
                    nc.tensor.matmul(
                        ps,
                        wlp_sb[:, hc * KC + kc, :],
                        seqi_sb[:, hc, :],
                        start=(hc == 0),
                        stop=(hc == KC - 1),
                    )
                # Lt = left_proj + b1 (bias folded here, fp32)
                nc.vector.tensor_scalar(
                    lt_sb[:, kc, :], ps, relb1_sb[:, kc : kc + 1], None, ADD
                )
                # rounded copy for the DVE max-trick and its exact correction
                nc.vector.tensor_copy(lt_bf[:, kc, :], lt_sb[:, kc, :])
                nc.vector.tensor_scalar(
                    lt_ng[:, kc, :], lt_bf[:, kc, :], -1.0, None,
                    mybir.AluOpType.mult,
                )

            # ---- correction term C[i,o] = sum_k w2[k,o]*l[i,k] over the
            # chunk prefixes the DVE units cover (4 or 5 chunks) ----
            ps_c = head_psum.tile([NIPC, 32], f32, tag="hp", name="ps_c")
            for kc in range(4):
                nc.tensor.matmul(
                    ps_c, lt_bf[:, kc, :], w2_sb[:, kc, :],
                    start=(kc == 0), stop=(kc == 3),
                )
            co4_sb = rl_pool.tile([NIPC, 32], f32, name="co4_sb")
            nc.vector.tensor_copy(co4_sb, ps_c)
            ps_c2 = head_psum.tile([NIPC, 32], f32, tag="hp", name="ps_c2")
            nc.tensor.matmul(ps_c2, lt_bf[:, 4, :], w2_sb[:, 4, :], start=True, stop=True)
            co5_sb = rl_pool.tile([NIPC, 32], f32, name="co5_sb")
            nc.vector.tensor_add(co5_sb, co4_sb, ps_c2)
            nc.sync.dma_start(co4[:, :], co4_sb)
            nc.sync.dma_start(co5[:, :], co5_sb)

            # ---- main pair loop ----
            for ig in range(NG):
                ps = pair_psum.tile([P, L], f32, tag="pp", name="ps_pair")
                kc_dve = KC - _akc(ig)   # DVE covers chunk prefix, ACT the tail
                bal.t["dve"] += kc_dve * 4 * _DVE_UNIT
                bal.t["act"] += (KC - kc_dve) * 4 * _ACT_UNIT
                for kc in range(KC):
                    for g in range(4):
                        i = ig * 4 + g
                        if kc < kc_dve:
                            h = hpool.tile([P, L], pdt, tag="hD", bufs=10, name="hd")
                            nc.vector.tensor_scalar(
                                h, rt_sb[:, kc, :], lt_ng[:, kc, i : i + 1], None, MAX
                            )
                        else:
                            h = hpool.tile([P, L], pdt, tag="hA", bufs=16, name="ha")
                            nc.scalar.activation(
                                h, rt_sb[:, kc, :], RELU, bias=lt_sb[:, kc, i : i + 1]
                            )
                        tp = (0, 32 * g) if COL_TILING else None
                        # skip_group_check: the sim's zero-region tracker is
                        # partition-unaware; the four col-groups accumulate into
                        # disjoint 32-partition slices of this bank.
                        nc.tensor.matmul(
                            ps[32 * g : 32 * g + 32, :],
                            w2_sb[:, kc, :],
                            h,
                            start=(kc == 0),
                            stop=(kc == KC - 1),
                            tile_position=tp,
                            skip_group_check=True,
                        )
                ob = opool.tile([P, L], f32, tag="ob", name="ob")
                eng = bal.pick(_DVE_COPY, _ACT_COPY)
                if eng == "dve":
                    nc.vector.tensor_copy(ob, ps)
                else:
                    nc.scalar.copy(ob, ps)
                nc.sync.dma_start(relo[ig, :, :], ob)

            # ---- checkmark head ----
            ckhid = rl_pool.tile([P, 3, P], mdt, name="ckhid")
            for mt in range(3):
                ps = head_psum.tile([P, P], f32, tag="hp", name="ps_ck")
                for hc in range(KC):
                    nc.tensor.matmul(
                        ps,
                        ckw1_sb[:, hc * 3 + mt, :],
                        seqi_sb[:, hc, :],
                        start=(hc == 0),
                        stop=(hc == KC - 1),
                    )
                nc.vector.tensor_scalar(
                    ckhid[:, mt, :], ps, ckb1_sb[:, mt : mt + 1], 0.0, ADD, MAX
                )
            ps_cko = head_psum.tile([4, NIPC], f32, tag="hp", name="ps_cko")
            for mc in range(3):
                nc.tensor.matmul(
                    ps_cko,
                    ckw2_sb[:, mc, :],
                    ckhid[:, mc, :],
                    start=(mc == 0),
                    stop=(mc == 2),
                )
            cko_sb = rl_pool.tile([4, NIPC], f32, name="cko_sb")
            nc.vector.tensor_copy(cko_sb, ps_cko)
            nc.sync.dma_start(cko[:, :], cko_sb)

            # ---- reasoning head on CLS ----
            rsfeat = rl_pool.tile([P, KC, 1], mdt, name="rsfeat")
            for mc in range(KC):
                ps = head_psum.tile([P, 1], f32, tag="hp", name="ps_rs")
                for hc in range(KC):
                    nc.tensor.matmul(
                        ps,
                        rsw1_sb[:, hc * KC + mc, :],
                        cls_sb[:, hc, :],
                        start=(hc == 0),
                        stop=(hc == KC - 1),
                    )
                nc.vector.tensor_scalar(
                    rsfeat[:, mc, :], ps, rsb1_sb[:, mc : mc + 1], 0.0, ADD, MAX
                )
            ps_rso = head_psum.tile([8, 1], f32, tag="hp", name="ps_rso")
            for mc in range(KC):
                nc.tensor.matmul(
                    ps_rso,
                    rsw2_sb[:, mc, :],
                    rsfeat[:, mc, :],
                    start=(mc == 0),
                    stop=(mc == KC - 1),
                )
            rso_sb = rl_pool.tile([8, 1], f32, name="rso_sb")
            nc.vector.tensor_copy(rso_sb, ps_rso)
            nc.sync.dma_start(rso[:, :], rso_sb)

            # ---- solution head on concat(cls, rsfeat) ----
            slhid = rl_pool.tile([P, KC, 1], mdt, name="slhid")
            for mc in range(KC):
                ps = head_psum.tile([P, 1], f32, tag="hp", name="ps_sl")
                for qc in range(2 * KC):
                    rhs = cls_sb[:, qc, :] if qc < KC else rsfeat[:, qc - KC, :]
                    nc.tensor.matmul(
                        ps,
                        slw1_sb[:, qc * KC + mc, :],
                        rhs,
                        start=(qc == 0),
                        stop=(qc == 2 * KC - 1),
                    )
                nc.vector.tensor_scalar(
                    slhid[:, mc, :], ps, slb1_sb[:, mc : mc + 1], 0.0, ADD, MAX
                )
            ps_slo = head_psum.tile([8, 1], f32, tag="hp", name="ps_slo")
            for mc in range(KC):
                nc.tensor.matmul(
                    ps_slo,
                    slw2_sb[:, mc, :],
                    slhid[:, mc, :],
                    start=(mc == 0),
                    stop=(mc == KC - 1),
                )
            slo_sb = rl_pool.tile([8, 1], f32, name="slo_sb")
            nc.vector.tensor_copy(slo_sb, ps_slo)
            nc.sync.dma_start(slo[:, :], slo_sb)

    nc.finalize()
    return nc


_PROGRAM = None


def _get_program():
    global _PROGRAM
    if _PROGRAM is None:
        _PROGRAM = _build_program()
    return _PROGRAM


def _pnp():
    if PAIR_BF16:
        import ml_dtypes

        return ml_dtypes.bfloat16
    return np.float32


def _r6(a):
    """(KC*P, X...) -> (P, KC, X...) partition-major."""
    a = np.ascontiguousarray(a)
    nchunk = a.shape[0] // P
    out = a.reshape(nchunk, P, *a.shape[1:]).swapaxes(0, 1)
    return np.ascontiguousarray(out)


def _wchunks(w):
    """(nh*P, nm*P) -> (P, nh*nm, P): [p, h_c*nm + m_c, m] = w[h_c*P+p, m_c*P+m]."""
    nh = w.shape[0] // P
    nm = w.shape[1] // P
    out = w.reshape(nh, P, nm, P).transpose(1, 0, 2, 3).reshape(P, nh * nm, P)
    return np.ascontiguousarray(out)


def make_in_maps(
    sequence_output, rel_w1, rel_b1, rel_w2,
    chk_w1, chk_b1, chk_w2,
    rsn_w1, rsn_b1, rsn_w2,
    sol_w1, sol_b1, sol_w2,
):
    pnp = _pnp()
    wl, wr, wd = rel_w1[:H], rel_w1[H : 2 * H], rel_w1[2 * H :]
    wlp = _wchunks((wl + wd).astype(np.float32)).astype(pnp)
    wrp = _wchunks((wr - wd).astype(np.float32)).astype(pnp)
    relb1 = np.ascontiguousarray(rel_b1.reshape(KC, P).T.astype(np.float32))
    w2_pad = np.zeros((H, 32), np.float32)
    w2_pad[:, :3] = rel_w2.astype(np.float32)
    w2a = _r6(w2_pad).astype(pnp)
    ckw1a = _wchunks(chk_w1.astype(np.float32)).astype(pnp)
    ckb1a = np.ascontiguousarray(chk_b1.reshape(3, P).T.astype(np.float32))
    ckw2a = _r6(chk_w2.astype(np.float32)).astype(pnp)
    rsw1a = _wchunks(rsn_w1.astype(np.float32)).astype(pnp)
    rsb1a = np.ascontiguousarray(rsn_b1.reshape(KC, P).T.astype(np.float32))
    rsw2a = _r6(rsn_w2.astype(np.float32)).astype(pnp)
    slw1a = _wchunks(sol_w1.astype(np.float32)).astype(pnp)
    slb1a = np.ascontiguousarray(sol_b1.reshape(KC, P).T.astype(np.float32))
    slw2a = _r6(sol_w2.astype(np.float32)).astype(pnp)

    in_maps = []
    for c in range(NCORES):
        b, blk = divmod(c, 4)
        i0 = blk * NIPC
        seq_t = np.ascontiguousarray(sequence_output[b].T.astype(np.float32))  # (H, L)
        in_maps.append({
            "seq": _r6(seq_t).astype(pnp),
            "seqi": _r6(np.ascontiguousarray(seq_t[:, i0 : i0 + NIPC])).astype(pnp),
            "cls": _r6(seq_t[:, 0:1]).astype(pnp),
            "wlp": wlp,
            "wrp": wrp,
            "relb1": relb1,
            "w2": w2a,
            "ckw1": ckw1a,
            "ckb1": ckb1a,
            "ckw2": ckw2a,
            "rsw1": rsw1a,
            "rsb1": rsb1a,
            "rsw2": rsw2a,
            "slw1": slw1a,
            "slb1": slb1a,
            "slw2": slw2a,
        })
    return in_maps


def unshard(results, rel_b2, chk_b2, rsn_b2, sol_b2):
    rel = np.empty((B, L, L, 3), np.float32)
    chk = np.empty((B, L, 4), np.float32)
    rsn = np.empty((B, 8), np.float32)
    sol = np.empty((B, 8), np.float32)
    for c in range(NCORES):
        b, blk = divmod(c, 4)
        i0 = blk * NIPC
        r = results[c]
        v = np.asarray(r["relo"]).reshape(NG, 4, 32, L)[:, :, :3, :]
        blk_rel = v.transpose(0, 1, 3, 2).reshape(NIPC, L, 3)
        co4a = np.asarray(r["co4"])[:, :3]
        co5a = np.asarray(r["co5"])[:, :3]
        corr = np.empty((NIPC, 3), np.float32)
        for ig in range(NG):
            csel = co4a if _akc(ig) == 2 else co5a
            corr[ig * 4 : ig * 4 + 4] = csel[ig * 4 : ig * 4 + 4]
        blk_rel = blk_rel + corr[:, None, :]
        rel[b, i0 : i0 + NIPC] = blk_rel
        chk[b, i0 : i0 + NIPC] = np.asarray(r["cko"]).T
        if blk == 0:
            rsn[b] = np.asarray(r["rso"])[:, 0]
            sol[b] = np.asarray(r["slo"])[:, 0]
    rel += rel_b2.astype(np.float32)
    chk += chk_b2.astype(np.float32)
    rsn += rsn_b2.astype(np.float32)
    sol += sol_b2.astype(np.float32)
    return rel, chk, rsn, sol


def run_spmd(in_maps, **kwargs):
    from concourse.bass_utils import run_bass_kernel_spmd

    nc = _get_program()
    return run_bass_kernel_spmd(nc, in_maps, core_ids=list(range(NCORES)), **kwargs)


def kernel(**inputs):
    inputs = {k: np.asarray(v) for k, v in inputs.items()}
    in_maps = make_in_maps(
        inputs["sequence_output"],
        inputs["rel_w1"], inputs["rel_b1"], inputs["rel_w2"],
        inputs["chk_w1"], inputs["chk_b1"], inputs["chk_w2"],
        inputs["rsn_w1"], inputs["rsn_b1"], inputs["rsn_w2"],
        inputs["sol_w1"], inputs["sol_b1"], inputs["sol_w2"],
    )
    res = run_spmd(in_maps)
    return unshard(
        res.results,
        inputs["rel_b2"], inputs["chk_b2"], inputs["rsn_b2"], inputs["sol_b2"],
    )


# revision 27
# speedup vs baseline: 1.0611x; 1.0611x over previous
"""Trainium2 Bass kernel for EnhancedLiLTRelationExtraction.

Shapes: B=2, L=512, H=768.
Outputs: rel_logits (B,L,L,3), checkmark (B,L,4), reasoning (B,8), solution (B,8).

Sharding: 8 cores = batch(2) x left-token-blocks(4).  Each core computes a
(128, 512) block of the pair grid plus the per-token checkmark head for its
token slice; cores 0 and 4 also carry the (tiny) reasoning/solution heads
for their batch.

Per-core pair path (the heavy part):
  Rt[k, j] = right_proj(b)[j, k]       (H on partitions, 6 chunks of 128)
  Lt[k, i] = left_proj(b)[i, k] + b1   (fp32, feeds per-partition scalar)
  for each left token i:  h = relu(Rt + Lt[:, i])  (one fused DVE/ACT op
  per k-chunk), then PE contracts h with rel_w2 chunk into PSUM,
  4 left tokens concurrently via column tiling (tile_position).

All matmul operands are bf16 (PSUM accumulation fp32).  DVE producer units
use the identity relu(r+l) = max(r,-l) + l: the max is a single-op
tensor_scalar, and the Sum_k w2*l correction (rank-1, j-independent) is
computed exactly on-device as C = Lt^T w2 (outputs co4/co5) and added back
on the host during unshard.  ACT units compute relu(r+l) directly and need
no correction; ACT takes whole trailing k-chunks (1 or 2, alternating by
block) so the correction per token is a fixed chunk-prefix sum.
"""

import sys

if "/opt/trn_rl_repo" not in sys.path:
    sys.path.insert(0, "/opt/trn_rl_repo")

import numpy as np

B, L, H = 2, 512, 768
P = 128
KC = H // P            # 6 k-chunks
NIPC = L // 4          # 128 left tokens per core
NG = NIPC // 4         # 32 groups of 4 left tokens
NCORES = 8

# Pair-loop matmul operands (Rt, h, w2) in bf16: required for PE column
# tiling (fp32/f32r weights use 4 physical PE columns per logical column, so
# their matmul destination must start at PSUM partition 0) and enables the
# DVE 4x perf mode for the relu producer.  Projections/heads stay float32r.
PAIR_BF16 = True
# 4-way PE column tiling for the pair contraction matmuls.
COL_TILING = True
# benchmarking only: trace the whole kernel body this many times in one NEFF
REPEAT = 1

# engine cost estimates (ns, HW-measured) for static DVE/ACT load balancing
_DVE_UNIT = 262
_ACT_UNIT = 627
_DVE_COPY = 658
_ACT_COPY = 570


def _akc(ig):
    """number of trailing k-chunks ACT computes for block ig (1 or 2).

    Measured costs: DVE max-unit ~262ns, ACT relu-unit ~627ns.  4-of-7
    blocks at 2 chunks gives ACT ~6.3 of 24 units per block, balancing
    both engines once ACT also absorbs most PSUM-evacuation copies."""
    return 2 if (ig % 7) < 4 else 1


class _Balancer:
    def __init__(self):
        self.t = {"dve": 0.0, "act": 0.0}

    def pick(self, dve_cost, act_cost):
        if self.t["dve"] + dve_cost <= self.t["act"] + act_cost:
            self.t["dve"] += dve_cost
            return "dve"
        self.t["act"] += act_cost
        return "act"


def _build_program():
    import concourse.mybir as mybir
    from concourse import bacc
    from concourse.tile import TileContext

    f32 = mybir.dt.float32
    mdt = mybir.dt.bfloat16
    pdt = mybir.dt.bfloat16
    ADD = mybir.AluOpType.add
    MAX = mybir.AluOpType.max
    RELU = mybir.ActivationFunctionType.Relu

    nc = bacc.Bacc("TRN2", target_bir_lowering=False)

    # ---- inputs (all host-side pre-rearranged to partition-major) ----
    seq = nc.dram_tensor("seq", [P, KC, L], mdt, kind="ExternalInput")
    seqi = nc.dram_tensor("seqi", [P, KC, NIPC], mdt, kind="ExternalInput")
    cls = nc.dram_tensor("cls", [P, KC, 1], mdt, kind="ExternalInput")
    wlp = nc.dram_tensor("wlp", [P, KC * KC, P], mdt, kind="ExternalInput")
    wrp = nc.dram_tensor("wrp", [P, KC * KC, P], mdt, kind="ExternalInput")
    relb1 = nc.dram_tensor("relb1", [P, KC], f32, kind="ExternalInput")
    # rel_w2 padded from 3 to 32 output columns with zeros so each PE column
    # group writes its full 32-partition PSUM slice (no uninitialized reads).
    w2 = nc.dram_tensor("w2", [P, KC, 32], pdt, kind="ExternalInput")
    ckw1 = nc.dram_tensor("ckw1", [P, KC * 3, P], mdt, kind="ExternalInput")
    ckb1 = nc.dram_tensor("ckb1", [P, 3], f32, kind="ExternalInput")
    ckw2 = nc.dram_tensor("ckw2", [P, 3, 4], mdt, kind="ExternalInput")
    rsw1 = nc.dram_tensor("rsw1", [P, KC * KC, P], mdt, kind="ExternalInput")
    rsb1 = nc.dram_tensor("rsb1", [P, KC], f32, kind="ExternalInput")
    rsw2 = nc.dram_tensor("rsw2", [P, KC, 8], mdt, kind="ExternalInput")
    slw1 = nc.dram_tensor("slw1", [P, 2 * KC * KC, P], mdt, kind="ExternalInput")
    slb1 = nc.dram_tensor("slb1", [P, KC], f32, kind="ExternalInput")
    slw2 = nc.dram_tensor("slw2", [P, KC, 8], mdt, kind="ExternalInput")

    # ---- outputs ----
    relo = nc.dram_tensor("relo", [NG, P, L], f32, kind="ExternalOutput")
    cko = nc.dram_tensor("cko", [4, NIPC], f32, kind="ExternalOutput")
    rso = nc.dram_tensor("rso", [8, 1], f32, kind="ExternalOutput")
    slo = nc.dram_tensor("slo", [8, 1], f32, kind="ExternalOutput")
    co4 = nc.dram_tensor("co4", [NIPC, 32], f32, kind="ExternalOutput")
    co5 = nc.dram_tensor("co5", [NIPC, 32], f32, kind="ExternalOutput")

    with TileContext(nc) as tc:
      for _rep in range(REPEAT):
        bal = _Balancer()
        with (
            tc.tile_pool(name="const", bufs=1) as cpool,
            tc.tile_pool(name="rtlt", bufs=1) as rl_pool,
            tc.tile_pool(name="hpool", bufs=12) as hpool,
            tc.tile_pool(name="opool", bufs=4) as opool,
            tc.tile_pool(name="ppsum", bufs=2, space="PSUM") as ppsum,
            tc.tile_pool(name="pair_psum", bufs=4, space="PSUM") as pair_psum,
            tc.tile_pool(name="head_psum", bufs=2, space="PSUM") as head_psum,
        ):
            # ---- load constants ----
            def load(name, shape, dtype, src):
                t = cpool.tile(shape, dtype, name=name)
                nc.sync.dma_start(t, src)
                return t

            seq_sb = load("seq_sb", [P, KC, L], mdt, seq[:, :, :])
            seqi_sb = load("seqi_sb", [P, KC, NIPC], mdt, seqi[:, :, :])
            cls_sb = load("cls_sb", [P, KC, 1], mdt, cls[:, :, :])
            wlp_sb = load("wlp_sb", [P, KC * KC, P], mdt, wlp[:, :, :])
            wrp_sb = load("wrp_sb", [P, KC * KC, P], mdt, wrp[:, :, :])
            relb1_sb = load("relb1_sb", [P, KC], f32, relb1[:, :])
            w2_sb = load("w2_sb", [P, KC, 32], pdt, w2[:, :, :])
            ckw1_sb = load("ckw1_sb", [P, KC * 3, P], mdt, ckw1[:, :, :])
            ckb1_sb = load("ckb1_sb", [P, 3], f32, ckb1[:, :])
            ckw2_sb = load("ckw2_sb", [P, 3, 4], mdt, ckw2[:, :, :])
            rsw1_sb = load("rsw1_sb", [P, KC * KC, P], mdt, rsw1[:, :, :])
            rsb1_sb = load("rsb1_sb", [P, KC], f32, rsb1[:, :])
            rsw2_sb = load("rsw2_sb", [P, KC, 8], mdt, rsw2[:, :, :])
            slw1_sb = load("slw1_sb", [P, 2 * KC * KC, P], mdt, slw1[:, :, :])
            slb1_sb = load("slb1_sb", [P, KC], f32, slb1[:, :])
            slw2_sb = load("slw2_sb", [P, KC, 8], mdt, slw2[:, :, :])

            # ---- projections: Rt (k-part, j) and Lt (k-part, i) ----
            rt_sb = rl_pool.tile([P, KC, L], pdt, name="rt_sb")
            lt_sb = rl_pool.tile([P, KC, NIPC], f32, name="lt_sb")
            lt_bf = rl_pool.tile([P, KC, NIPC], pdt, name="lt_bf")
            lt_ng = rl_pool.tile([P, KC, NIPC], f32, name="lt_ng")
            for kc in range(KC):
                ps = ppsum.tile([P, L], f32, tag="proj", name="ps_rt")
                for hc in range(KC):
                    nc.tensor.matmul(
                        ps,
                        wrp_sb[:, hc * KC + kc, :],
                        seq_sb[:, hc, :],
                        start=(hc == 0),
                        stop=(hc == KC - 1),
                    )
                eng = bal.pick(_DVE_COPY, _ACT_COPY)
                if eng == "dve":
                    nc.vector.tensor_copy(rt_sb[:, kc, :], ps)
                else:
                    nc.scalar.copy(rt_sb[:, kc, :], ps)
            for kc in range(KC):
                ps = ppsum.tile([P, NIPC], f32, tag="proj", name="ps_lt")
                for hc in range(KC):
                    nc.tensor.matmul(
                        ps,
                        wlp_sb[:, hc * KC + kc, :],
                        seqi_sb[:, hc, :],
                        start=(hc == 0),
                        stop=(hc == KC - 1),
                    )
                # Lt = left_proj + b1 (bias folded here, fp32)
                nc.vector.tensor_scalar(
                    lt_sb[:, kc, :], ps, relb1_sb[:, kc : kc + 1], None, ADD
                )
                # rounded copy for the DVE max-trick and its exact correction
                nc.vector.tensor_copy(lt_bf[:, kc, :], lt_sb[:, kc, :])
                nc.vector.tensor_scalar(
                    lt_ng[:, kc, :], lt_bf[:, kc, :], -1.0, None,
                    mybir.AluOpType.mult,
                )

            # ---- correction term C[i,o] = sum_k w2[k,o]*l[i,k] over the
            # chunk prefixes the DVE units cover (4 or 5 chunks) ----
            ps_c = head_psum.tile([NIPC, 32], f32, tag="hp", name="ps_c")
            for kc in range(4):
                nc.tensor.matmul(
                    ps_c, lt_bf[:, kc, :], w2_sb[:, kc, :],
                    start=(kc == 0), stop=(kc == 3),
                )
            co4_sb = rl_pool.tile([NIPC, 32], f32, name="co4_sb")
            nc.vector.tensor_copy(co4_sb, ps_c)
            ps_c2 = head_psum.tile([NIPC, 32], f32, tag="hp", name="ps_c2")
            nc.tensor.matmul(ps_c2, lt_bf[:, 4, :], w2_sb[:, 4, :], start=True, stop=True)
            co5_sb = rl_pool.tile([NIPC, 32], f32, name="co5_sb")
            nc.vector.tensor_add(co5_sb, co4_sb, ps_c2)
            nc.sync.dma_start(co4[:, :], co4_sb)
            nc.sync.dma_start(co5[:, :], co5_sb)

            # ---- main pair loop ----
            for ig in range(NG):
                ps = pair_psum.tile([P, L], f32, tag="pp", name="ps_pair")
                kc_dve = KC - _akc(ig)   # DVE covers chunk prefix, ACT the tail
                bal.t["dve"] += kc_dve * 4 * _DVE_UNIT
                bal.t["act"] += (KC - kc_dve) * 4 * _ACT_UNIT
                for kc in range(KC):
                    for g in range(4):
                        i = ig * 4 + g
                        if kc < kc_dve:
                            h = hpool.tile([P, L], pdt, tag="hD", bufs=10, name="hd")
                            nc.vector.tensor_scalar(
                                h, rt_sb[:, kc, :], lt_ng[:, kc, i : i + 1], None, MAX
                            )
                        else:
                            h = hpool.tile([P, L], pdt, tag="hA", bufs=16, name="ha")
                            nc.scalar.activation(
                                h, rt_sb[:, kc, :], RELU, bias=lt_sb[:, kc, i : i + 1]
                            )
                        tp = (0, 32 * g) if COL_TILING else None
                        # skip_group_check: the sim's zero-region tracker is
                        # partition-unaware; the four col-groups accumulate into
                        # disjoint 32-partition slices of this bank.
                        nc.tensor.matmul(
                            ps[32 * g : 32 * g + 32, :],
                            w2_sb[:, kc, :],
                            h,
                            start=(kc == 0),
                            stop=(kc == KC - 1),
                            tile_position=tp,
                            skip_group_check=True,
                        )
                ob = opool.tile([P, L], f32, tag="ob", name="ob")
                eng = bal.pick(_DVE_COPY, _ACT_COPY)
                if eng == "dve":
                    nc.vector.tensor_copy(ob, ps)
                else:
                    nc.scalar.copy(ob, ps)
                nc.sync.dma_start(relo[ig, :, :], ob)

            # ---- checkmark head ----
            ckhid = rl_pool.tile([P, 3, P], mdt, name="ckhid")
            for mt in range(3):
                ps = head_psum.tile([P, P], f32, tag="hp", name="ps_ck")
                for hc in range(KC):
                    nc.tensor.matmul(
                        ps,
                        ckw1_sb[:, hc * 3 + mt, :],
                        seqi_sb[:, hc, :],
                        start=(hc == 0),
                        stop=(hc == KC - 1),
                    )
                nc.vector.tensor_scalar(
                    ckhid[:, mt, :], ps, ckb1_sb[:, mt : mt + 1], 0.0, ADD, MAX
                )
            ps_cko = head_psum.tile([4, NIPC], f32, tag="hp", name="ps_cko")
            for mc in range(3):
                nc.tensor.matmul(
                    ps_cko,
                    ckw2_sb[:, mc, :],
                    ckhid[:, mc, :],
                    start=(mc == 0),
                    stop=(mc == 2),
                )
            cko_sb = rl_pool.tile([4, NIPC], f32, name="cko_sb")
            nc.vector.tensor_copy(cko_sb, ps_cko)
            nc.sync.dma_start(cko[:, :], cko_sb)

            # ---- reasoning head on CLS ----
            rsfeat = rl_pool.tile([P, KC, 1], mdt, name="rsfeat")
            for mc in range(KC):
                ps = head_psum.tile([P, 1], f32, tag="hp", name="ps_rs")
                for hc in range(KC):
                    nc.tensor.matmul(
                        ps,
                        rsw1_sb[:, hc * KC + mc, :],
                        cls_sb[:, hc, :],
                        start=(hc == 0),
                        stop=(hc == KC - 1),
                    )
                nc.vector.tensor_scalar(
                    rsfeat[:, mc, :], ps, rsb1_sb[:, mc : mc + 1], 0.0, ADD, MAX
                )
            ps_rso = head_psum.tile([8, 1], f32, tag="hp", name="ps_rso")
            for mc in range(KC):
                nc.tensor.matmul(
                    ps_rso,
                    rsw2_sb[:, mc, :],
                    rsfeat[:, mc, :],
                    start=(mc == 0),
                    stop=(mc == KC - 1),
                )
            rso_sb = rl_pool.tile([8, 1], f32, name="rso_sb")
            nc.vector.tensor_copy(rso_sb, ps_rso)
            nc.sync.dma_start(rso[:, :], rso_sb)

            # ---- solution head on concat(cls, rsfeat) ----
            slhid = rl_pool.tile([P, KC, 1], mdt, name="slhid")
            for mc in range(KC):
                ps = head_psum.tile([P, 1], f32, tag="hp", name="ps_sl")
                for qc in range(2 * KC):
                    rhs = cls_sb[:, qc, :] if qc < KC else rsfeat[:, qc - KC, :]
                    nc.tensor.matmul(
                        ps,
                        slw1_sb[:, qc * KC + mc, :],
                        rhs,
                        start=(qc == 0),
                        stop=(qc == 2 * KC - 1),
                    )
                nc.vector.tensor_scalar(
                    slhid[:, mc, :], ps, slb1_sb[:, mc : mc + 1], 0.0, ADD, MAX
                )
            ps_slo = head_psum.tile([8, 1], f32, tag="hp", name="ps_slo")
            for mc in range(KC):
                nc.tensor.matmul(
                    ps_slo,
                    slw2_sb[:, mc, :],
                    slhid[:, mc, :],
                    start=(mc == 0),
                    stop=(mc == KC - 1),
                )
            slo_sb = rl_pool.tile([8, 1], f32, name="slo_sb")
            nc.vector.tensor_copy(slo_sb, ps_slo)
            nc.sync.dma_start(slo[:, :], slo_sb)

    nc.finalize()
    return nc


_PROGRAM = None


def _get_program():
    global _PROGRAM
    if _PROGRAM is None:
        _PROGRAM = _build_program()
    return _PROGRAM


def _pnp():
    if PAIR_BF16:
        import ml_dtypes

        return ml_dtypes.bfloat16
    return np.float32


def _r6(a):
    """(KC*P, X...) -> (P, KC, X...) partition-major."""
    a = np.ascontiguousarray(a)
    nchunk = a.shape[0] // P
    out = a.reshape(nchunk, P, *a.shape[1:]).swapaxes(0, 1)
    return np.ascontiguousarray(out)


def _wchunks(w):
    """(nh*P, nm*P) -> (P, nh*nm, P): [p, h_c*nm + m_c, m] = w[h_c*P+p, m_c*P+m]."""
    nh = w.shape[0] // P
    nm = w.shape[1] // P
    out = w.reshape(nh, P, nm, P).transpose(1, 0, 2, 3).reshape(P, nh * nm, P)
    return np.ascontiguousarray(out)


def make_in_maps(
    sequence_output, rel_w1, rel_b1, rel_w2,
    chk_w1, chk_b1, chk_w2,
    rsn_w1, rsn_b1, rsn_w2,
    sol_w1, sol_b1, sol_w2,
):
    pnp = _pnp()
    wl, wr, wd = rel_w1[:H], rel_w1[H : 2 * H], rel_w1[2 * H :]
    wlp = _wchunks((wl + wd).astype(np.float32)).astype(pnp)
    wrp = _wchunks((wr - wd).astype(np.float32)).astype(pnp)
    relb1 = np.ascontiguousarray(rel_b1.reshape(KC, P).T.astype(np.float32))
    w2_pad = np.zeros((H, 32), np.float32)
    w2_pad[:, :3] = rel_w2.astype(np.float32)
    w2a = _r6(w2_pad).astype(pnp)
    ckw1a = _wchunks(chk_w1.astype(np.float32)).astype(pnp)
    ckb1a = np.ascontiguousarray(chk_b1.reshape(3, P).T.astype(np.float32))
    ckw2a = _r6(chk_w2.astype(np.float32)).astype(pnp)
    rsw1a = _wchunks(rsn_w1.astype(np.float32)).astype(pnp)
    rsb1a = np.ascontiguousarray(rsn_b1.reshape(KC, P).T.astype(np.float32))
    rsw2a = _r6(rsn_w2.astype(np.float32)).astype(pnp)
    slw1a = _wchunks(sol_w1.astype(np.float32)).astype(pnp)
    slb1a = np.ascontiguousarray(sol_b1.reshape(KC, P).T.astype(np.float32))
    slw2a = _r6(sol_w2.astype(np.float32)).astype(pnp)

    in_maps = []
    for c in range(NCORES):
        b, blk = divmod(c, 4)
        i0 = blk * NIPC
        seq_t = np.ascontiguousarray(sequence_output[b].T.astype(np.float32))  # (H, L)
        in_maps.append({
            "seq": _r6(seq_t).astype(pnp),
            "seqi": _r6(np.ascontiguousarray(seq_t[:, i0 : i0 + NIPC])).astype(pnp),
            "cls": _r6(seq_t[:, 0:1]).astype(pnp),
            "wlp": wlp,
            "wrp": wrp,
            "relb1": relb1,
            "w2": w2a,
            "ckw1": ckw1a,
            "ckb1": ckb1a,
            "ckw2": ckw2a,
            "rsw1": rsw1a,
            "rsb1": rsb1a,
            "rsw2": rsw2a,
            "slw1": slw1a,
            "slb1": slb1a,
            "slw2": slw2a,
        })
    return in_maps


def unshard(results, rel_b2, chk_b2, rsn_b2, sol_b2):
    rel = np.empty((B, L, L, 3), np.float32)
    chk = np.empty((B, L, 4), np.float32)
    rsn = np.empty((B, 8), np.float32)
    sol = np.empty((B, 8), np.float32)
    for c in range(NCORES):
        b, blk = divmod(c, 4)
        i0 = blk * NIPC
        r = results[c]
        v = np.asarray(r["relo"]).reshape(NG, 4, 32, L)[:, :, :3, :]
        blk_rel = v.transpose(0, 1, 3, 2).reshape(NIPC, L, 3)
        co4a = np.asarray(r["co4"])[:, :3]
        co5a = np.asarray(r["co5"])[:, :3]
        corr = np.empty((NIPC, 3), np.float32)
        for ig in range(NG):
            csel = co4a if _akc(ig) == 2 else co5a
            corr[ig * 4 : ig * 4 + 4] = csel[ig * 4 : ig * 4 + 4]
        blk_rel = blk_rel + corr[:, None, :]
        rel[b, i0 : i0 + NIPC] = blk_rel
        chk[b, i0 : i0 + NIPC] = np.asarray(r["cko"]).T
        if blk == 0:
            rsn[b] = np.asarray(r["rso"])[:, 0]
            sol[b] = np.asarray(r["slo"])[:, 0]
    rel += rel_b2.astype(np.float32)
    chk += chk_b2.astype(np.float32)
    rsn += rsn_b2.astype(np.float32)
    sol += sol_b2.astype(np.float32)
    return rel, chk, rsn, sol


def run_spmd(in_maps, **kwargs):
    from concourse.bass_utils import run_bass_kernel_spmd

    nc = _get_program()
    return run_bass_kernel_spmd(nc, in_maps, core_ids=list(range(NCORES)), **kwargs)


def kernel(**inputs):
    inputs = {k: np.asarray(v) for k, v in inputs.items()}
    in_maps = make_in_maps(
        inputs["sequence_output"],
        inputs["rel_w1"], inputs["rel_b1"], inputs["rel_w2"],
        inputs["chk_w1"], inputs["chk_b1"], inputs["chk_w2"],
        inputs["rsn_w1"], inputs["rsn_b1"], inputs["rsn_w2"],
        inputs["sol_w1"], inputs["sol_b1"], inputs["sol_w2"],
    )
    res = run_spmd(in_maps)
    return unshard(
        res.results,
        inputs["rel_b2"], inputs["chk_b2"], inputs["rsn_b2"], inputs["sol_b2"],
    )


# revision 28
# speedup vs baseline: 1.0673x; 1.0058x over previous
"""Trainium2 Bass kernel for EnhancedLiLTRelationExtraction.

Shapes: B=2, L=512, H=768.
Outputs: rel_logits (B,L,L,3), checkmark (B,L,4), reasoning (B,8), solution (B,8).

Sharding: 8 cores = batch(2) x left-token-blocks(4).  Each core computes a
(128, 512) block of the pair grid plus the per-token checkmark head for its
token slice; cores 0 and 4 also carry the (tiny) reasoning/solution heads
for their batch.

Per-core pair path (the heavy part):
  Rt[k, j] = right_proj(b)[j, k]       (H on partitions, 6 chunks of 128)
  Lt[k, i] = left_proj(b)[i, k] + b1   (fp32, feeds per-partition scalar)
  for each left token i:  h = relu(Rt + Lt[:, i])  (one fused DVE/ACT op
  per k-chunk), then PE contracts h with rel_w2 chunk into PSUM,
  4 left tokens concurrently via column tiling (tile_position).

All matmul operands are bf16 (PSUM accumulation fp32).  DVE producer units
use the identity relu(r+l) = max(r,-l) + l: the max is a single-op
tensor_scalar, and the Sum_k w2*l correction (rank-1, j-independent) is
computed exactly on-device as C = Lt^T w2 (outputs co4/co5) and added back
on the host during unshard.  ACT units compute relu(r+l) directly and need
no correction; ACT takes whole trailing k-chunks (1 or 2, alternating by
block) so the correction per token is a fixed chunk-prefix sum.
"""

import sys

if "/opt/trn_rl_repo" not in sys.path:
    sys.path.insert(0, "/opt/trn_rl_repo")

import numpy as np

B, L, H = 2, 512, 768
P = 128
KC = H // P            # 6 k-chunks
NIPC = L // 4          # 128 left tokens per core
NG = NIPC // 4         # 32 groups of 4 left tokens
NCORES = 8

# Pair-loop matmul operands (Rt, h, w2) in bf16: required for PE column
# tiling (fp32/f32r weights use 4 physical PE columns per logical column, so
# their matmul destination must start at PSUM partition 0) and enables the
# DVE 4x perf mode for the relu producer.  Projections/heads stay float32r.
PAIR_BF16 = True
# 4-way PE column tiling for the pair contraction matmuls.
COL_TILING = True
# benchmarking only: trace the whole kernel body this many times in one NEFF
REPEAT = 1

# engine cost estimates (ns, HW-measured) for static DVE/ACT load balancing
_DVE_UNIT = 262
_ACT_UNIT = 627
_DVE_COPY = 658
_ACT_COPY = 570


def _akc(ig):
    """number of trailing k-chunks ACT computes for block ig (1 or 2).

    Measured costs: DVE max-unit ~262ns, ACT relu-unit ~627ns.  4-of-7
    blocks at 2 chunks gives ACT ~6.3 of 24 units per block, balancing
    both engines once ACT also absorbs most PSUM-evacuation copies."""
    return 2 if (ig % 7) < 4 else 1


class _Balancer:
    def __init__(self):
        self.t = {"dve": 0.0, "act": 0.0}

    def pick(self, dve_cost, act_cost):
        if self.t["dve"] + dve_cost <= self.t["act"] + act_cost:
            self.t["dve"] += dve_cost
            return "dve"
        self.t["act"] += act_cost
        return "act"


def _build_program():
    import concourse.mybir as mybir
    from concourse import bacc
    from concourse.tile import TileContext

    f32 = mybir.dt.float32
    mdt = mybir.dt.bfloat16
    pdt = mybir.dt.bfloat16
    ADD = mybir.AluOpType.add
    MAX = mybir.AluOpType.max
    RELU = mybir.ActivationFunctionType.Relu

    nc = bacc.Bacc("TRN2", target_bir_lowering=False)

    # ---- inputs (all host-side pre-rearranged to partition-major) ----
    seq = nc.dram_tensor("seq", [P, KC, L], mdt, kind="ExternalInput")
    seqi = nc.dram_tensor("seqi", [P, KC, NIPC], mdt, kind="ExternalInput")
    cls = nc.dram_tensor("cls", [P, KC, 1], mdt, kind="ExternalInput")
    wlp = nc.dram_tensor("wlp", [P, KC * KC, P], mdt, kind="ExternalInput")
    wrp = nc.dram_tensor("wrp", [P, KC * KC, P], mdt, kind="ExternalInput")
    relb1 = nc.dram_tensor("relb1", [P, KC], f32, kind="ExternalInput")
    # rel_w2 padded from 3 to 32 output columns with zeros so each PE column
    # group writes its full 32-partition PSUM slice (no uninitialized reads).
    w2 = nc.dram_tensor("w2", [P, KC, 32], pdt, kind="ExternalInput")
    ckw1 = nc.dram_tensor("ckw1", [P, KC * 3, P], mdt, kind="ExternalInput")
    ckb1 = nc.dram_tensor("ckb1", [P, 3], f32, kind="ExternalInput")
    ckw2 = nc.dram_tensor("ckw2", [P, 3, 4], mdt, kind="ExternalInput")
    rsw1 = nc.dram_tensor("rsw1", [P, KC * KC, P], mdt, kind="ExternalInput")
    rsb1 = nc.dram_tensor("rsb1", [P, KC], f32, kind="ExternalInput")
    rsw2 = nc.dram_tensor("rsw2", [P, KC, 8], mdt, kind="ExternalInput")
    slw1 = nc.dram_tensor("slw1", [P, 2 * KC * KC, P], mdt, kind="ExternalInput")
    slb1 = nc.dram_tensor("slb1", [P, KC], f32, kind="ExternalInput")
    slw2 = nc.dram_tensor("slw2", [P, KC, 8], mdt, kind="ExternalInput")

    # ---- outputs ----
    relo = nc.dram_tensor("relo", [NG, P, L], f32, kind="ExternalOutput")
    cko = nc.dram_tensor("cko", [4, NIPC], f32, kind="ExternalOutput")
    rso = nc.dram_tensor("rso", [8, 1], f32, kind="ExternalOutput")
    slo = nc.dram_tensor("slo", [8, 1], f32, kind="ExternalOutput")
    co4 = nc.dram_tensor("co4", [NIPC, 32], f32, kind="ExternalOutput")
    co5 = nc.dram_tensor("co5", [NIPC, 32], f32, kind="ExternalOutput")

    with TileContext(nc) as tc:
      for _rep in range(REPEAT):
        bal = _Balancer()
        with (
            tc.tile_pool(name="const", bufs=1) as cpool,
            tc.tile_pool(name="rtlt", bufs=1) as rl_pool,
            tc.tile_pool(name="hpool", bufs=12) as hpool,
            tc.tile_pool(name="opool", bufs=4) as opool,
            tc.tile_pool(name="ppsum", bufs=1, space="PSUM") as ppsum,
            tc.tile_pool(name="pair_psum", bufs=4, space="PSUM") as pair_psum,
            tc.tile_pool(name="head_psum", bufs=2, space="PSUM") as head_psum,
        ):
            # ---- load constants ----
            def load(name, shape, dtype, src):
                t = cpool.tile(shape, dtype, name=name)
                nc.sync.dma_start(t, src)
                return t

            # projection-critical tensors arrive as per-chunk DMAs so the
            # first Rt/Lt chunks (and thus the pair producers) start early.
            seq_sb = cpool.tile([P, KC, L], mdt, name="seq_sb")
            seqi_sb = cpool.tile([P, KC, NIPC], mdt, name="seqi_sb")
            wlp_sb = cpool.tile([P, KC * KC, P], mdt, name="wlp_sb")
            wrp_sb = cpool.tile([P, KC * KC, P], mdt, name="wrp_sb")
            for hc in range(KC):
                nc.sync.dma_start(seq_sb[:, hc, :], seq[:, hc, :])
            nc.sync.dma_start(wrp_sb[:, 0:KC, :], wrp[:, 0:KC, :])
            for hc in range(KC):
                nc.sync.dma_start(seqi_sb[:, hc, :], seqi[:, hc, :])
            nc.sync.dma_start(wlp_sb[:, 0:KC, :], wlp[:, 0:KC, :])
            for kc in range(1, KC):
                nc.sync.dma_start(
                    wrp_sb[:, kc * KC : (kc + 1) * KC, :],
                    wrp[:, kc * KC : (kc + 1) * KC, :],
                )
                nc.sync.dma_start(
                    wlp_sb[:, kc * KC : (kc + 1) * KC, :],
                    wlp[:, kc * KC : (kc + 1) * KC, :],
                )
            relb1_sb = load("relb1_sb", [P, KC], f32, relb1[:, :])
            w2_sb = load("w2_sb", [P, KC, 32], pdt, w2[:, :, :])

            # ---- projections: Rt (k-part, j) and Lt (k-part, i) ----
            rt_sb = rl_pool.tile([P, KC, L], pdt, name="rt_sb")
            lt_sb = rl_pool.tile([P, KC, NIPC], f32, name="lt_sb")
            lt_bf = rl_pool.tile([P, KC, NIPC], pdt, name="lt_bf")
            lt_ng = rl_pool.tile([P, KC, NIPC], f32, name="lt_ng")
            for kc in range(KC):
                ps = ppsum.tile([P, L], f32, tag="proj", name="ps_rt")
                for hc in range(KC):
                    nc.tensor.matmul(
                        ps,
                        wrp_sb[:, kc * KC + hc, :],
                        seq_sb[:, hc, :],
                        start=(hc == 0),
                        stop=(hc == KC - 1),
                    )
                eng = bal.pick(_DVE_COPY, _ACT_COPY)
                if eng == "dve":
                    nc.vector.tensor_copy(rt_sb[:, kc, :], ps)
                else:
                    nc.scalar.copy(rt_sb[:, kc, :], ps)
                ps = ppsum.tile([P, NIPC], f32, tag="proj2", name="ps_lt")
                for hc in range(KC):
                    nc.tensor.matmul(
                        ps,
                        wlp_sb[:, kc * KC + hc, :],
                        seqi_sb[:, hc, :],
                        start=(hc == 0),
                        stop=(hc == KC - 1),
                    )
                # Lt = left_proj + b1 (bias folded here, fp32)
                nc.vector.tensor_scalar(
                    lt_sb[:, kc, :], ps, relb1_sb[:, kc : kc + 1], None, ADD
                )
                # rounded copy for the DVE max-trick and its exact correction
                nc.vector.tensor_copy(lt_bf[:, kc, :], lt_sb[:, kc, :])
                nc.vector.tensor_scalar(
                    lt_ng[:, kc, :], lt_bf[:, kc, :], -1.0, None,
                    mybir.AluOpType.mult,
                )

            # head weights are only needed late; load after projections
            cls_sb = load("cls_sb", [P, KC, 1], mdt, cls[:, :, :])
            ckw1_sb = load("ckw1_sb", [P, KC * 3, P], mdt, ckw1[:, :, :])
            ckb1_sb = load("ckb1_sb", [P, 3], f32, ckb1[:, :])
            ckw2_sb = load("ckw2_sb", [P, 3, 4], mdt, ckw2[:, :, :])
            rsw1_sb = load("rsw1_sb", [P, KC * KC, P], mdt, rsw1[:, :, :])
            rsb1_sb = load("rsb1_sb", [P, KC], f32, rsb1[:, :])
            rsw2_sb = load("rsw2_sb", [P, KC, 8], mdt, rsw2[:, :, :])
            slw1_sb = load("slw1_sb", [P, 2 * KC * KC, P], mdt, slw1[:, :, :])
            slb1_sb = load("slb1_sb", [P, KC], f32, slb1[:, :])
            slw2_sb = load("slw2_sb", [P, KC, 8], mdt, slw2[:, :, :])

            # ---- correction term C[i,o] = sum_k w2[k,o]*l[i,k] over the
            # chunk prefixes the DVE units cover (4 or 5 chunks) ----
            ps_c = head_psum.tile([NIPC, 32], f32, tag="hp", name="ps_c")
            for kc in range(4):
                nc.tensor.matmul(
                    ps_c, lt_bf[:, kc, :], w2_sb[:, kc, :],
                    start=(kc == 0), stop=(kc == 3),
                )
            co4_sb = rl_pool.tile([NIPC, 32], f32, name="co4_sb")
            nc.vector.tensor_copy(co4_sb, ps_c)
            ps_c2 = head_psum.tile([NIPC, 32], f32, tag="hp", name="ps_c2")
            nc.tensor.matmul(ps_c2, lt_bf[:, 4, :], w2_sb[:, 4, :], start=True, stop=True)
            co5_sb = rl_pool.tile([NIPC, 32], f32, name="co5_sb")
            nc.vector.tensor_add(co5_sb, co4_sb, ps_c2)
            nc.sync.dma_start(co4[:, :], co4_sb)
            nc.sync.dma_start(co5[:, :], co5_sb)

            # ---- main pair loop ----
            for ig in range(NG):
                ps = pair_psum.tile([P, L], f32, tag="pp", name="ps_pair")
                kc_dve = KC - _akc(ig)   # DVE covers chunk prefix, ACT the tail
                bal.t["dve"] += kc_dve * 4 * _DVE_UNIT
                bal.t["act"] += (KC - kc_dve) * 4 * _ACT_UNIT
                for kc in range(KC):
                    for g in range(4):
                        i = ig * 4 + g
                        if kc < kc_dve:
                            h = hpool.tile([P, L], pdt, tag="hD", bufs=10, name="hd")
                            nc.vector.tensor_scalar(
                                h, rt_sb[:, kc, :], lt_ng[:, kc, i : i + 1], None, MAX
                            )
                        else:
                            h = hpool.tile([P, L], pdt, tag="hA", bufs=16, name="ha")
                            nc.scalar.activation(
                                h, rt_sb[:, kc, :], RELU, bias=lt_sb[:, kc, i : i + 1]
                            )
                        tp = (0, 32 * g) if COL_TILING else None
                        # skip_group_check: the sim's zero-region tracker is
                        # partition-unaware; the four col-groups accumulate into
                        # disjoint 32-partition slices of this bank.
                        nc.tensor.matmul(
                            ps[32 * g : 32 * g + 32, :],
                            w2_sb[:, kc, :],
                            h,
                            start=(kc == 0),
                            stop=(kc == KC - 1),
                            tile_position=tp,
                            skip_group_check=True,
                        )
                ob = opool.tile([P, L], f32, tag="ob", name="ob")
                eng = bal.pick(_DVE_COPY, _ACT_COPY)
                if eng == "dve":
                    nc.vector.tensor_copy(ob, ps)
                else:
                    nc.scalar.copy(ob, ps)
                nc.sync.dma_start(relo[ig, :, :], ob)

            # ---- checkmark head ----
            ckhid = rl_pool.tile([P, 3, P], mdt, name="ckhid")
            for mt in range(3):
                ps = head_psum.tile([P, P], f32, tag="hp", name="ps_ck")
                for hc in range(KC):
                    nc.tensor.matmul(
                        ps,
                        ckw1_sb[:, hc * 3 + mt, :],
                        seqi_sb[:, hc, :],
                        start=(hc == 0),
                        stop=(hc == KC - 1),
                    )
                nc.vector.tensor_scalar(
                    ckhid[:, mt, :], ps, ckb1_sb[:, mt : mt + 1], 0.0, ADD, MAX
                )
            ps_cko = head_psum.tile([4, NIPC], f32, tag="hp", name="ps_cko")
            for mc in range(3):
                nc.tensor.matmul(
                    ps_cko,
                    ckw2_sb[:, mc, :],
                    ckhid[:, mc, :],
                    start=(mc == 0),
                    stop=(mc == 2),
                )
            cko_sb = rl_pool.tile([4, NIPC], f32, name="cko_sb")
            nc.vector.tensor_copy(cko_sb, ps_cko)
            nc.sync.dma_start(cko[:, :], cko_sb)

            # ---- reasoning head on CLS ----
            rsfeat = rl_pool.tile([P, KC, 1], mdt, name="rsfeat")
            for mc in range(KC):
                ps = head_psum.tile([P, 1], f32, tag="hp", name="ps_rs")
                for hc in range(KC):
                    nc.tensor.matmul(
                        ps,
                        rsw1_sb[:, hc * KC + mc, :],
                        cls_sb[:, hc, :],
                        start=(hc == 0),
                        stop=(hc == KC - 1),
                    )
                nc.vector.tensor_scalar(
                    rsfeat[:, mc, :], ps, rsb1_sb[:, mc : mc + 1], 0.0, ADD, MAX
                )
            ps_rso = head_psum.tile([8, 1], f32, tag="hp", name="ps_rso")
            for mc in range(KC):
                nc.tensor.matmul(
                    ps_rso,
                    rsw2_sb[:, mc, :],
                    rsfeat[:, mc, :],
                    start=(mc == 0),
                    stop=(mc == KC - 1),
                )
            rso_sb = rl_pool.tile([8, 1], f32, name="rso_sb")
            nc.vector.tensor_copy(rso_sb, ps_rso)
            nc.sync.dma_start(rso[:, :], rso_sb)

            # ---- solution head on concat(cls, rsfeat) ----
            slhid = rl_pool.tile([P, KC, 1], mdt, name="slhid")
            for mc in range(KC):
                ps = head_psum.tile([P, 1], f32, tag="hp", name="ps_sl")
                for qc in range(2 * KC):
                    rhs = cls_sb[:, qc, :] if qc < KC else rsfeat[:, qc - KC, :]
                    nc.tensor.matmul(
                        ps,
                        slw1_sb[:, qc * KC + mc, :],
                        rhs,
                        start=(qc == 0),
                        stop=(qc == 2 * KC - 1),
                    )
                nc.vector.tensor_scalar(
                    slhid[:, mc, :], ps, slb1_sb[:, mc : mc + 1], 0.0, ADD, MAX
                )
            ps_slo = head_psum.tile([8, 1], f32, tag="hp", name="ps_slo")
            for mc in range(KC):
                nc.tensor.matmul(
                    ps_slo,
                    slw2_sb[:, mc, :],
                    slhid[:, mc, :],
                    start=(mc == 0),
                    stop=(mc == KC - 1),
                )
            slo_sb = rl_pool.tile([8, 1], f32, name="slo_sb")
            nc.vector.tensor_copy(slo_sb, ps_slo)
            nc.sync.dma_start(slo[:, :], slo_sb)

    nc.finalize()
    return nc


_PROGRAM = None


def _get_program():
    global _PROGRAM
    if _PROGRAM is None:
        _PROGRAM = _build_program()
    return _PROGRAM


def _pnp():
    if PAIR_BF16:
        import ml_dtypes

        return ml_dtypes.bfloat16
    return np.float32


def _r6(a):
    """(KC*P, X...) -> (P, KC, X...) partition-major."""
    a = np.ascontiguousarray(a)
    nchunk = a.shape[0] // P
    out = a.reshape(nchunk, P, *a.shape[1:]).swapaxes(0, 1)
    return np.ascontiguousarray(out)


def _wchunks(w):
    """(nh*P, nm*P) -> (P, nh*nm, P): [p, h_c*nm + m_c, m] = w[h_c*P+p, m_c*P+m]."""
    nh = w.shape[0] // P
    nm = w.shape[1] // P
    out = w.reshape(nh, P, nm, P).transpose(1, 0, 2, 3).reshape(P, nh * nm, P)
    return np.ascontiguousarray(out)


def _wchunks_kcmajor(w):
    """(nh*P, nm*P) -> (P, nm*nh, P): [p, m_c*nh + h_c, m] = w[h_c*P+p, m_c*P+m].

    kc-major so each output chunk's weights are one contiguous DMA slice."""
    nh = w.shape[0] // P
    nm = w.shape[1] // P
    out = w.reshape(nh, P, nm, P).transpose(1, 2, 0, 3).reshape(P, nm * nh, P)
    return np.ascontiguousarray(out)


def make_in_maps(
    sequence_output, rel_w1, rel_b1, rel_w2,
    chk_w1, chk_b1, chk_w2,
    rsn_w1, rsn_b1, rsn_w2,
    sol_w1, sol_b1, sol_w2,
):
    pnp = _pnp()
    wl, wr, wd = rel_w1[:H], rel_w1[H : 2 * H], rel_w1[2 * H :]
    wlp = _wchunks_kcmajor((wl + wd).astype(np.float32)).astype(pnp)
    wrp = _wchunks_kcmajor((wr - wd).astype(np.float32)).astype(pnp)
    relb1 = np.ascontiguousarray(rel_b1.reshape(KC, P).T.astype(np.float32))
    w2_pad = np.zeros((H, 32), np.float32)
    w2_pad[:, :3] = rel_w2.astype(np.float32)
    w2a = _r6(w2_pad).astype(pnp)
    ckw1a = _wchunks(chk_w1.astype(np.float32)).astype(pnp)
    ckb1a = np.ascontiguousarray(chk_b1.reshape(3, P).T.astype(np.float32))
    ckw2a = _r6(chk_w2.astype(np.float32)).astype(pnp)
    rsw1a = _wchunks(rsn_w1.astype(np.float32)).astype(pnp)
    rsb1a = np.ascontiguousarray(rsn_b1.reshape(KC, P).T.astype(np.float32))
    rsw2a = _r6(rsn_w2.astype(np.float32)).astype(pnp)
    slw1a = _wchunks(sol_w1.astype(np.float32)).astype(pnp)
    slb1a = np.ascontiguousarray(sol_b1.reshape(KC, P).T.astype(np.float32))
    slw2a = _r6(sol_w2.astype(np.float32)).astype(pnp)

    in_maps = []
    for c in range(NCORES):
        b, blk = divmod(c, 4)
        i0 = blk * NIPC
        seq_t = np.ascontiguousarray(sequence_output[b].T.astype(np.float32))  # (H, L)
        in_maps.append({
            "seq": _r6(seq_t).astype(pnp),
            "seqi": _r6(np.ascontiguousarray(seq_t[:, i0 : i0 + NIPC])).astype(pnp),
            "cls": _r6(seq_t[:, 0:1]).astype(pnp),
            "wlp": wlp,
            "wrp": wrp,
            "relb1": relb1,
            "w2": w2a,
            "ckw1": ckw1a,
            "ckb1": ckb1a,
            "ckw2": ckw2a,
            "rsw1": rsw1a,
            "rsb1": rsb1a,
            "rsw2": rsw2a,
            "slw1": slw1a,
            "slb1": slb1a,
            "slw2": slw2a,
        })
    return in_maps


def unshard(results, rel_b2, chk_b2, rsn_b2, sol_b2):
    rel = np.empty((B, L, L, 3), np.float32)
    chk = np.empty((B, L, 4), np.float32)
    rsn = np.empty((B, 8), np.float32)
    sol = np.empty((B, 8), np.float32)
    for c in range(NCORES):
        b, blk = divmod(c, 4)
        i0 = blk * NIPC
        r = results[c]
        v = np.asarray(r["relo"]).reshape(NG, 4, 32, L)[:, :, :3, :]
        blk_rel = v.transpose(0, 1, 3, 2).reshape(NIPC, L, 3)
        co4a = np.asarray(r["co4"])[:, :3]
        co5a = np.asarray(r["co5"])[:, :3]
        corr = np.empty((NIPC, 3), np.float32)
        for ig in range(NG):
            csel = co4a if _akc(ig) == 2 else co5a
            corr[ig * 4 : ig * 4 + 4] = csel[ig * 4 : ig * 4 + 4]
        blk_rel = blk_rel + corr[:, None, :]
        rel[b, i0 : i0 + NIPC] = blk_rel
        chk[b, i0 : i0 + NIPC] = np.asarray(r["cko"]).T
        if blk == 0:
            rsn[b] = np.asarray(r["rso"])[:, 0]
            sol[b] = np.asarray(r["slo"])[:, 0]
    rel += rel_b2.astype(np.float32)
    chk += chk_b2.astype(np.float32)
    rsn += rsn_b2.astype(np.float32)
    sol += sol_b2.astype(np.float32)
    return rel, chk, rsn, sol


def run_spmd(in_maps, **kwargs):
    from concourse.bass_utils import run_bass_kernel_spmd

    nc = _get_program()
    return run_bass_kernel_spmd(nc, in_maps, core_ids=list(range(NCORES)), **kwargs)


def kernel(**inputs):
    inputs = {k: np.asarray(v) for k, v in inputs.items()}
    in_maps = make_in_maps(
        inputs["sequence_output"],
        inputs["rel_w1"], inputs["rel_b1"], inputs["rel_w2"],
        inputs["chk_w1"], inputs["chk_b1"], inputs["chk_w2"],
        inputs["rsn_w1"], inputs["rsn_b1"], inputs["rsn_w2"],
        inputs["sol_w1"], inputs["sol_b1"], inputs["sol_w2"],
    )
    res = run_spmd(in_maps)
    return unshard(
        res.results,
        inputs["rel_b2"], inputs["chk_b2"], inputs["rsn_b2"], inputs["sol_b2"],
    )
